# revision 8
# baseline (speedup 1.0000x reference)
"""MoE FeedForward (dMoE) Trainium2 kernel: 8-core expert-parallel SPMD.

Sharding (hardcoded from the problem spec / sharding hint):
  - T=8192 tokens, D=1024, F=4096, 7 routed experts (top-2, capacity 2926) + 1
    shared expert.  Core c (c<7) owns routed expert c; core 7 runs a dummy
    expert (zero weights, no members).  Every core additionally computes the
    shared expert on tokens [c*1024, (c+1)*1024).
  - The router is replicated: each core computes LN stats + fp32 router logits
    for all tokens (router matmuls use a host-pretransposed copy of x as the
    stationary operand; LN enters as an exact linear correction), top-2 with
    renormalized sigmoid gates, then capacity-buffer slot positions via a
    free-axis prefix scan plus a triangular-matmul cross-partition offset pass.
  - Dispatch: a slot->(token, gate) table is built with an indirect scatter
    (bounds-check skips non-members and capacity drops; drops zero the gate),
    then expert GEMM inputs are fetched with indirect row gathers.
  - The expert FFN runs in bf16 with fp32 PSUM accumulation (the router stays
    fp32 so top-2 selection matches the fp32 reference).
  - Combine: routed outputs scatter into a pre-zeroed per-core partial output
    (rows are globally distinct per core, so plain writes suffice); the shared
    slice is a dense per-core tensor.  Host sums partials and adds the slices.
"""

import sys
import types
import numpy as np
import ml_dtypes

P = 128
T = 8192
D = 1024
F = 4096
ER = 7
CAP = 2926             # reference capacity
NSLOT_R = 2944         # padded routed slots (23 * 128)
NSH = 1024             # shared tokens per core
NT = T // P            # 64 token tiles
BIG = float(1 << 24)
N_CORES = 8
EPS = 1e-5

_cached = {}


def _install_ntff_shim():
    """bass_utils wants antenv.axon_hooks for trace=True; provide it if absent."""
    try:
        import antenv.axon_hooks  # noqa: F401
        return
    except ImportError:
        pass
    try:
        from trn_agent_boot.trn_boot import _ntff_profile_via_ctypes
        hook = _ntff_profile_via_ctypes('/opt/axon/libaxon_pjrt.so')
    except Exception:
        hook = None
    mod = types.ModuleType("antenv.axon_hooks")
    mod.get_axon_ntff_profile_hook = lambda: hook
    mod.set_axon_ntff_profile_hook = lambda h: None
    sys.modules["antenv.axon_hooks"] = mod


def build_nc():
    import concourse.bass as bass
    import concourse.mybir as mybir
    import concourse.tile as tile
    from concourse import bacc
    from concourse.masks import make_identity

    f32, bf16, i32, u8 = (mybir.dt.float32, mybir.dt.bfloat16,
                          mybir.dt.int32, mybir.dt.uint8)
    AF = mybir.ActivationFunctionType
    ALU = mybir.AluOpType
    AX = mybir.AxisListType
    IOA = bass.IndirectOffsetOnAxis

    nc = bacc.Bacc(None, target_bir_lowering=False)

    # ---------------- DRAM I/O ----------------
    x_in = nc.dram_tensor("x_in", [T, D], f32, kind="ExternalInput")
    xt_in = nc.dram_tensor("xt_in", [NT, P, D], f32, kind="ExternalInput")
    wr_in = nc.dram_tensor("wr_in", [P, ER * 8], f32, kind="ExternalInput")
    wsum_in = nc.dram_tensor("wsum_in", [P, ER], f32, kind="ExternalInput")
    g_in = nc.dram_tensor("g_in", [P, D], f32, kind="ExternalInput")
    b_in = nc.dram_tensor("b_in", [P, D], f32, kind="ExternalInput")
    tri_in = nc.dram_tensor("tri_in", [P, P], f32, kind="ExternalInput")
    eid_in = nc.dram_tensor("eid_in", [P, 1], f32, kind="ExternalInput")
    sh0_in = nc.dram_tensor("sh0_in", [P, 1], f32, kind="ExternalInput")
    w1_in = nc.dram_tensor("w1_in", [D, F], bf16, kind="ExternalInput")
    w3_in = nc.dram_tensor("w3_in", [D, F], bf16, kind="ExternalInput")
    w2_in = nc.dram_tensor("w2_in", [F, D], bf16, kind="ExternalInput")
    w1s_in = nc.dram_tensor("w1s_in", [D, F], bf16, kind="ExternalInput")
    w3s_in = nc.dram_tensor("w3s_in", [D, F], bf16, kind="ExternalInput")
    w2s_in = nc.dram_tensor("w2s_in", [F, D], bf16, kind="ExternalInput")

    partial = nc.dram_tensor("partial", [T, D], f32, kind="ExternalOutput")
    y_shared = nc.dram_tensor("y_shared", [NSH, D], f32, kind="ExternalOutput")

    # internal DRAM
    h_dram = nc.dram_tensor("h_dram", [T, D], bf16)
    meta_dram = nc.dram_tensor("meta_dram", [NSLOT_R, 2], f32)
    gt_dram = nc.dram_tensor("gt_dram", [F, NSLOT_R], bf16)
    gts_dram = nc.dram_tensor("gts_dram", [F, NSH], bf16)

    from contextlib import ExitStack
    with tile.TileContext(nc) as tc, ExitStack() as _stk:
        cpool = _stk.enter_context(tc.tile_pool(name="consts", bufs=1))
        ident = cpool.tile([P, P], f32)
        make_identity(nc, ident[:])
        identb = cpool.tile([P, P], bf16)
        make_identity(nc, identb[:])
        tri = cpool.tile([P, P], f32)
        nc.sync.dma_start(out=tri[:], in_=tri_in[:])
        wr_sb = cpool.tile([P, ER * 8], f32)
        nc.sync.dma_start(out=wr_sb[:], in_=wr_in[:])
        wsum = cpool.tile([P, ER], f32)
        nc.sync.dma_start(out=wsum[:], in_=wsum_in[:])
        g_rep = cpool.tile([P, D], f32)
        nc.sync.dma_start(out=g_rep[:], in_=g_in[:])
        b_rep = cpool.tile([P, D], f32)
        nc.sync.dma_start(out=b_rep[:], in_=b_in[:])
        eid = cpool.tile([P, 1], f32)
        nc.sync.dma_start(out=eid[:], in_=eid_in[:])
        sh0 = cpool.tile([P, 1], f32)
        nc.sync.dma_start(out=sh0[:], in_=sh0_in[:])
        io7i = cpool.tile([P, ER], i32)
        nc.gpsimd.iota(io7i[:], pattern=[[1, ER]], base=0, channel_multiplier=0)
        io7 = cpool.tile([P, ER], f32)
        nc.vector.tensor_copy(out=io7[:], in_=io7i[:])
        io128i = cpool.tile([P, 1], i32)
        nc.gpsimd.iota(io128i[:], pattern=[[0, 1]], base=0, channel_multiplier=1)
        io128 = cpool.tile([P, 1], f32)
        nc.vector.tensor_copy(out=io128[:], in_=io128i[:])
        big7 = cpool.tile([P, ER], f32)
        nc.vector.memset(big7[:], 99.0)
        low7 = cpool.tile([P, ER], f32)
        nc.vector.memset(low7[:], -1e30)
        big1 = cpool.tile([P, 1], f32)
        nc.vector.memset(big1[:], BIG)
        bigmeta = cpool.tile([P, 2], f32)
        nc.vector.memset(bigmeta[:], BIG)
        epst = cpool.tile([P, 1], f32)
        nc.vector.memset(epst[:], EPS)

        # meta defaults: BIG token ids so empty slots are skipped at combine
        for i in range(NSLOT_R // P):
            nc.sync.dma_start(out=meta_dram[i * P:(i + 1) * P, :], in_=bigmeta[:])

        with tc.tile_pool(name="state", bufs=1) as spool, \
             tc.tile_pool(name="passA", bufs=3) as apool, \
             tc.tile_pool(name="smalls", bufs=4) as small, \
             tc.tile_pool(name="pslog", bufs=2, space="PSUM") as pslog, \
             tc.tile_pool(name="pstr", bufs=2, space="PSUM") as pstr:
            memb_all = spool.tile([P, P], f32)
            nc.vector.memset(memb_all[:], 0.0)
            gate_all = spool.tile([P, NT], f32)
            pos_all = spool.tile([P, P], f32)

            # ---------------- Pass A: LN + router + top-2 ----------------
            for ti in range(NT):
                x_t = apool.tile([P, D], f32, tag="x")
                nc.sync.dma_start(out=x_t[:], in_=x_in[ti * P:(ti + 1) * P, :])
                xt_sb = apool.tile([P, D], f32, tag="xt")
                nc.sync.dma_start(out=xt_sb[:], in_=xt_in[ti, :, :])

                ssum = small.tile([P, 1], f32, tag="ssum")
                nc.vector.tensor_reduce(out=ssum[:], in_=x_t[:], axis=AX.X,
                                        op=ALU.add)
                mu = small.tile([P, 1], f32, tag="mu")
                nc.vector.tensor_scalar_mul(mu[:], ssum[:], 1.0 / D)
                xc = apool.tile([P, D], f32, tag="xc")
                nc.vector.tensor_scalar(out=xc[:], in0=x_t[:], scalar1=mu[:],
                                        scalar2=None, op0=ALU.subtract)
                sq = apool.tile([P, D], f32, tag="sq")
                var = small.tile([P, 1], f32, tag="var")
                nc.scalar.activation(out=sq[:], in_=xc[:], func=AF.Square,
                                     accum_out=var[:])
                std = small.tile([P, 1], f32, tag="std")
                nc.scalar.activation(out=std[:], in_=var[:], func=AF.Sqrt,
                                     scale=1.0 / D, bias=epst[:])
                rstd = small.tile([P, 1], f32, tag="rstd")
                nc.vector.reciprocal(out=rstd[:], in_=std[:])

                hs = apool.tile([P, D], f32, tag="hs")
                nc.vector.tensor_scalar(out=hs[:], in0=xc[:], scalar1=rstd[:],
                                        scalar2=None, op0=ALU.mult)
                hg = apool.tile([P, D], f32, tag="hg")
                nc.vector.tensor_tensor(out=hg[:], in0=hs[:], in1=g_rep[:],
                                        op=ALU.mult)
                h_bf = apool.tile([P, D], bf16, tag="h")
                nc.vector.tensor_tensor(out=h_bf[:], in0=hg[:], in1=b_rep[:],
                                        op=ALU.add)
                nc.sync.dma_start(out=h_dram[ti * P:(ti + 1) * P, :], in_=h_bf[:])

                # router: raw logits then exact LN linear correction
                ps_l = pslog.tile([P, ER], f32, space="PSUM")
                for k in range(8):
                    nc.tensor.matmul(out=ps_l[:],
                                     lhsT=xt_sb[:, k * P:(k + 1) * P],
                                     rhs=wr_sb[:, k * ER:(k + 1) * ER],
                                     start=(k == 0), stop=(k == 7))
                lcorr = small.tile([P, ER], f32, tag="lcorr")
                nc.vector.tensor_scalar(out=lcorr[:], in0=wsum[:], scalar1=mu[:],
                                        scalar2=None, op0=ALU.mult)
                lraw = small.tile([P, ER], f32, tag="lraw")
                nc.vector.tensor_tensor(out=lraw[:], in0=ps_l[:], in1=lcorr[:],
                                        op=ALU.subtract)
                lg = small.tile([P, ER], f32, tag="lg")
                nc.vector.tensor_scalar(out=lg[:], in0=lraw[:], scalar1=rstd[:],
                                        scalar2=None, op0=ALU.mult)

                m1 = small.tile([P, 1], f32, tag="m1")
                nc.vector.tensor_reduce(out=m1[:], in_=lg[:], axis=AX.X,
                                        op=ALU.max)
                eq1 = small.tile([P, ER], u8, tag="eq1")
                nc.vector.tensor_tensor(out=eq1[:], in0=lg[:],
                                        in1=m1[:].to_broadcast([P, ER]),
                                        op=ALU.is_equal)
                sel1 = small.tile([P, ER], f32, tag="sel1")
                nc.vector.select(out=sel1[:], mask=eq1[:], on_true=io7[:],
                                 on_false=big7[:])
                i1 = small.tile([P, 1], f32, tag="i1")
                nc.vector.tensor_reduce(out=i1[:], in_=sel1[:], axis=AX.X,
                                        op=ALU.min)
                lg2 = small.tile([P, ER], f32, tag="lg2")
                nc.vector.select(out=lg2[:], mask=eq1[:], on_true=low7[:],
                                 on_false=lg[:])
                m2 = small.tile([P, 1], f32, tag="m2")
                nc.vector.tensor_reduce(out=m2[:], in_=lg2[:], axis=AX.X,
                                        op=ALU.max)
                eq2 = small.tile([P, ER], u8, tag="eq2")
                nc.vector.tensor_tensor(out=eq2[:], in0=lg2[:],
                                        in1=m2[:].to_broadcast([P, ER]),
                                        op=ALU.is_equal)
                sel2 = small.tile([P, ER], f32, tag="sel2")
                nc.vector.select(out=sel2[:], mask=eq2[:], on_true=io7[:],
                                 on_false=big7[:])
                i2 = small.tile([P, 1], f32, tag="i2")
                nc.vector.tensor_reduce(out=i2[:], in_=sel2[:], axis=AX.X,
                                        op=ALU.min)

                dlt = small.tile([P, 1], f32, tag="dlt")
                nc.vector.tensor_sub(out=dlt[:], in0=m1[:], in1=m2[:])
                g1 = small.tile([P, 1], f32, tag="g1")
                nc.scalar.activation(out=g1[:], in_=dlt[:], func=AF.Sigmoid)
                g2 = small.tile([P, 1], f32, tag="g2")
                nc.vector.tensor_scalar(out=g2[:], in0=g1[:], scalar1=-1.0,
                                        scalar2=-1.0, op0=ALU.mult,
                                        op1=ALU.subtract)

                mk1 = small.tile([P, 1], f32, tag="mk1")
                nc.vector.tensor_tensor(out=mk1[:], in0=i1[:], in1=eid[:],
                                        op=ALU.is_equal)
                mk2 = small.tile([P, 1], f32, tag="mk2")
                nc.vector.tensor_tensor(out=mk2[:], in0=i2[:], in1=eid[:],
                                        op=ALU.is_equal)
                nc.vector.tensor_tensor(out=memb_all[:, ti:ti + 1], in0=mk1[:],
                                        in1=mk2[:], op=ALU.add)
                gm1 = small.tile([P, 1], f32, tag="gm1")
                nc.vector.tensor_tensor(out=gm1[:], in0=g1[:], in1=mk1[:],
                                        op=ALU.mult)
                gm2 = small.tile([P, 1], f32, tag="gm2")
                nc.vector.tensor_tensor(out=gm2[:], in0=g2[:], in1=mk2[:],
                                        op=ALU.mult)
                nc.vector.tensor_tensor(out=gate_all[:, ti:ti + 1], in0=gm1[:],
                                        in1=gm2[:], op=ALU.add)

            # ---------------- scan: slot positions ----------------
            mt_ps = pstr.tile([P, P], f32, space="PSUM", tag="mt")
            nc.tensor.transpose(out=mt_ps[:], in_=memb_all[:], identity=ident[:])
            mt = spool.tile([P, P], f32)
            nc.vector.tensor_copy(out=mt[:], in_=mt_ps[:])
            mc = spool.tile([P, P], f32)
            nc.vector.tensor_tensor_scan(out=mc[:], data0=mt[:], data1=mt[:],
                                         initial=0.0, op0=ALU.add, op1=ALU.bypass)
            offs_ps = pstr.tile([P, 1], f32, space="PSUM", tag="offs")
            nc.tensor.matmul(out=offs_ps[:], lhsT=tri[:], rhs=mc[:, P - 1:P],
                             start=True, stop=True)
            offs = spool.tile([P, 1], f32)
            nc.vector.tensor_copy(out=offs[:], in_=offs_ps[:])
            posT = spool.tile([P, P], f32)
            nc.vector.tensor_scalar(out=posT[:], in0=mc[:], scalar1=offs[:],
                                    scalar2=None, op0=ALU.add)
            nc.vector.tensor_tensor(out=posT[:], in0=posT[:], in1=mt[:],
                                    op=ALU.subtract)
            pos_ps = pstr.tile([P, P], f32, space="PSUM", tag="pos")
            nc.tensor.transpose(out=pos_ps[:], in_=posT[:], identity=ident[:])
            nc.vector.tensor_copy(out=pos_all[:], in_=pos_ps[:])

            # ---------------- Pass B: meta scatter (dispatch table) --------
            for ti in range(NT):
                pcol = pos_all[:, ti:ti + 1]
                keep = small.tile([P, 1], f32, tag="keep")
                nc.vector.tensor_scalar(out=keep[:], in0=pcol, scalar1=float(CAP),
                                        scalar2=None, op0=ALU.is_lt)
                both = small.tile([P, 1], f32, tag="both")
                nc.vector.tensor_tensor(out=both[:], in0=keep[:],
                                        in1=memb_all[:, ti:ti + 1], op=ALU.mult)
                both8 = small.tile([P, 1], u8, tag="both8")
                nc.vector.tensor_copy(out=both8[:], in_=both[:])
                slotr = small.tile([P, 1], f32, tag="slotr")
                nc.vector.select(out=slotr[:], mask=both8[:], on_true=pcol,
                                 on_false=big1[:])
                slotr_i = small.tile([P, 1], i32, tag="slotri")
                nc.vector.tensor_copy(out=slotr_i[:], in_=slotr[:])
                metar = small.tile([P, 2], f32, tag="metar")
                nc.vector.tensor_scalar(out=metar[:, 0:1], in0=io128[:],
                                        scalar1=float(ti * P), scalar2=None,
                                        op0=ALU.add)
                nc.vector.tensor_tensor(out=metar[:, 1:2],
                                        in0=gate_all[:, ti:ti + 1], in1=keep[:],
                                        op=ALU.mult)
                nc.gpsimd.indirect_dma_start(
                    out=meta_dram[:],
                    out_offset=IOA(ap=slotr_i[:, :1], axis=0),
                    in_=metar[:], in_offset=None,
                    bounds_check=NSLOT_R - 1, oob_is_err=False)

        # ---------------- Phase C: expert FFN ----------------
        def chunks_of(ntiles):
            out, o = [], 0
            while o < ntiles:
                n = min(4, ntiles - o)
                out.append((o * P, n * P))
                o += n
            return out

        PH = [
            (w1_in, w3_in, w2_in, gt_dram, chunks_of(NSLOT_R // P), True),
            (w1s_in, w3s_in, w2s_in, gts_dram, chunks_of(NSH // P), False),
        ]

        def load_xe(xepool, small2, row0, routed):
            """Gather one [P, D] bf16 tile of expert input rows."""
            xe_t = xepool.tile([P, D], bf16, tag="xe")
            idx_i = small2.tile([P, 1], i32, tag="gidx")
            if routed:
                meta_t = small2.tile([P, 2], f32, tag="gmeta")
                nc.sync.dma_start(out=meta_t[:],
                                  in_=meta_dram[row0:row0 + P, :])
                nc.vector.tensor_copy(out=idx_i[:], in_=meta_t[:, 0:1])
            else:
                idx_f = small2.tile([P, 1], f32, tag="gidxf")
                nc.vector.tensor_scalar(out=idx_f[:], in0=io128[:],
                                        scalar1=float(row0), scalar2=sh0[:],
                                        op0=ALU.add, op1=ALU.add)
                nc.vector.tensor_copy(out=idx_i[:], in_=idx_f[:])
            nc.gpsimd.indirect_dma_start(
                out=xe_t[:], out_offset=None, in_=h_dram[:],
                in_offset=IOA(ap=idx_i[:, :1], axis=0),
                bounds_check=T - 1, oob_is_err=False)
            return xe_t

        # C1: g = silu(x@W1) * (x@W3), bounced to DRAM transposed
        for w1t, w3t, w2t, gdram, chl, routed in PH:
            with tc.tile_pool(name="wpool", bufs=1) as wpool, \
                 tc.tile_pool(name="xepool", bufs=3) as xepool, \
                 tc.tile_pool(name="small2", bufs=4) as small2, \
                 tc.tile_pool(name="xtp", bufs=2) as xtp, \
                 tc.tile_pool(name="gvec", bufs=3) as gvec, \
                 tc.tile_pool(name="psT", bufs=3, space="PSUM") as psT, \
                 tc.tile_pool(name="psA", bufs=2, space="PSUM") as psA, \
                 tc.tile_pool(name="psB", bufs=2, space="PSUM") as psB:
                w1b = [wpool.tile([P, F], bf16, tag=f"w1b{k}", name=f"w1b{k}") for k in range(8)]
                w3b = [wpool.tile([P, F], bf16, tag=f"w3b{k}", name=f"w3b{k}") for k in range(8)]
                for k in range(8):
                    nc.sync.dma_start(out=w1b[k][:], in_=w1t[k * P:(k + 1) * P, :])
                    nc.sync.dma_start(out=w3b[k][:], in_=w3t[k * P:(k + 1) * P, :])
                for row0, nrow in chl:
                    xeT = [xtp.tile([P, nrow], bf16, tag=f"xeT{k}", name=f"xeT{k}")
                           for k in range(8)]
                    for t4 in range(nrow // P):
                        xe_t = load_xe(xepool, small2, row0 + t4 * P, routed)
                        for k in range(8):
                            tps = psT.tile([P, P], bf16, space="PSUM", tag="tps")
                            nc.tensor.transpose(out=tps[:],
                                                in_=xe_t[:, k * P:(k + 1) * P],
                                                identity=identb[:])
                            nc.vector.tensor_copy(
                                out=xeT[k][:, t4 * P:(t4 + 1) * P], in_=tps[:])
                    for mf in range(F // P):
                        ps_a = psA.tile([P, nrow], f32, space="PSUM", tag="psa")
                        for k in range(8):
                            nc.tensor.matmul(out=ps_a[:],
                                             lhsT=w1b[k][:, mf * P:(mf + 1) * P],
                                             rhs=xeT[k][:],
                                             start=(k == 0), stop=(k == 7))
                        sil = gvec.tile([P, nrow], f32, tag="sil")
                        nc.scalar.activation(out=sil[:], in_=ps_a[:], func=AF.Silu)
                        ps_b = psB.tile([P, nrow], f32, space="PSUM", tag="psb")
                        for k in range(8):
                            nc.tensor.matmul(out=ps_b[:],
                                             lhsT=w3b[k][:, mf * P:(mf + 1) * P],
                                             rhs=xeT[k][:],
                                             start=(k == 0), stop=(k == 7))
                        g_t = gvec.tile([P, nrow], bf16, tag="gt")
                        nc.vector.tensor_tensor(out=g_t[:], in0=sil[:],
                                                in1=ps_b[:], op=ALU.mult)
                        nc.sync.dma_start(
                            out=gdram[mf * P:(mf + 1) * P, row0:row0 + nrow],
                            in_=g_t[:])

        # C2: y = g @ W2, transpose back, gate-scale, combine
        for w1t, w3t, w2t, gdram, chl, routed in PH:
            with tc.tile_pool(name="w2pool", bufs=1) as w2pool, \
                 tc.tile_pool(name="gin", bufs=2) as gin, \
                 tc.tile_pool(name="yout", bufs=2) as yout, \
                 tc.tile_pool(name="ysm", bufs=4) as ysm, \
                 tc.tile_pool(name="psY", bufs=2, space="PSUM") as psY, \
                 tc.tile_pool(name="psT2", bufs=3, space="PSUM") as psT2:
                w2b = [w2pool.tile([P, D], bf16, tag=f"w2b{k}", name=f"w2b{k}")
                       for k in range(32)]
                for k in range(32):
                    nc.sync.dma_start(out=w2b[k][:], in_=w2t[k * P:(k + 1) * P, :])
                for row0, nrow in chl:
                    gT = [gin.tile([P, nrow], bf16, tag=f"gT{k}", name=f"gT{k}")
                          for k in range(32)]
                    for k in range(32):
                        nc.sync.dma_start(out=gT[k][:],
                                          in_=gdram[k * P:(k + 1) * P,
                                                    row0:row0 + nrow])
                    y_sb = []
                    for md in range(8):
                        ps_y = psY.tile([P, nrow], f32, space="PSUM", tag="psy")
                        for k in range(32):
                            nc.tensor.matmul(out=ps_y[:],
                                             lhsT=w2b[k][:, md * P:(md + 1) * P],
                                             rhs=gT[k][:],
                                             start=(k == 0), stop=(k == 31))
                        ysb = yout.tile([P, nrow], f32, tag=f"ysb{md}")
                        nc.vector.tensor_copy(out=ysb[:], in_=ps_y[:])
                        y_sb.append(ysb)
                    for t4 in range(nrow // P):
                        yrow = yout.tile([P, D], f32, tag="yrow")
                        for md in range(8):
                            tps = psT2.tile([P, P], f32, space="PSUM", tag="tps2")
                            nc.tensor.transpose(
                                out=tps[:], in_=y_sb[md][:, t4 * P:(t4 + 1) * P],
                                identity=ident[:])
                            nc.vector.tensor_copy(
                                out=yrow[:, md * P:(md + 1) * P], in_=tps[:])
                        if routed:
                            meta_t = ysm.tile([P, 2], f32, tag="metat")
                            nc.sync.dma_start(
                                out=meta_t[:],
                                in_=meta_dram[row0 + t4 * P:row0 + (t4 + 1) * P,
                                              :])
                            tok_i = ysm.tile([P, 1], i32, tag="toki")
                            nc.vector.tensor_copy(out=tok_i[:], in_=meta_t[:, 0:1])
                            nc.vector.tensor_scalar(out=yrow[:], in0=yrow[:],
                                                    scalar1=meta_t[:, 1:2],
                                                    scalar2=None, op0=ALU.mult)
                            nc.gpsimd.indirect_dma_start(
                                out=partial[:],
                                out_offset=IOA(ap=tok_i[:, :1], axis=0),
                                in_=yrow[:], in_offset=None,
                                bounds_check=T - 1, oob_is_err=False)
                        else:
                            r0 = row0 + t4 * P
                            nc.sync.dma_start(out=y_shared[r0:r0 + P, :],
                                              in_=yrow[:])
    nc.compile()
    return nc


def _prep_inputs(x, ln_g, ln_b, Wr, W1, W3, W2, W1s, W3s, W2s):
    bf16 = ml_dtypes.bfloat16
    f32 = np.float32
    x = np.ascontiguousarray(np.asarray(x, f32).reshape(T, D))
    Wr = np.asarray(Wr, f32)
    # pretransposed router tiles: xt[ti, dl, k*128+p] = x[ti*128+p, k*128+dl]
    xt = np.ascontiguousarray(
        x.reshape(NT, P, 8, P).transpose(0, 3, 2, 1).reshape(NT, P, D))
    wr_t = np.ascontiguousarray(
        Wr.reshape(8, P, ER).reshape(8, P, ER).transpose(1, 0, 2).reshape(P, 8 * ER))
    wsum = np.tile(Wr.sum(0)[None, :], (P, 1)).astype(f32)
    g_rep = np.tile(np.asarray(ln_g, f32)[None, :], (P, 1))
    b_rep = np.tile(np.asarray(ln_b, f32)[None, :], (P, 1))
    tri = (np.arange(P)[:, None] < np.arange(P)[None, :]).astype(f32)

    W1 = np.asarray(W1, f32)
    W3 = np.asarray(W3, f32)
    W2 = np.asarray(W2, f32)
    zero1 = np.zeros((D, F), bf16)
    zero2 = np.zeros((F, D), bf16)
    w1s_b = np.asarray(W1s, f32).astype(bf16)
    w3s_b = np.asarray(W3s, f32).astype(bf16)
    w2s_b = np.asarray(W2s, f32).astype(bf16)

    in_maps = []
    for c in range(N_CORES):
        m = {
            "x_in": x, "xt_in": xt, "wr_in": wr_t, "wsum_in": wsum,
            "g_in": g_rep, "b_in": b_rep, "tri_in": tri,
            "eid_in": np.full((P, 1), float(c), f32),
            "sh0_in": np.full((P, 1), float(c * NSH), f32),
            "w1s_in": w1s_b, "w3s_in": w3s_b, "w2s_in": w2s_b,
        }
        if c < ER:
            m["w1_in"] = W1[c].astype(bf16)
            m["w3_in"] = W3[c].astype(bf16)
            m["w2_in"] = W2[c].astype(bf16)
        else:
            m["w1_in"] = zero1
            m["w3_in"] = zero1
            m["w2_in"] = zero2
        in_maps.append(m)
    return in_maps


def kernel(x, ln_g, ln_b, Wr, W1, W3, W2, W1s, W3s, W2s, _trace=False):
    _install_ntff_shim()
    from concourse.bass_utils import run_bass_kernel_spmd

    if "nc" not in _cached:
        _cached["nc"] = build_nc()
    nc = _cached["nc"]

    in_maps = _prep_inputs(x, ln_g, ln_b, Wr, W1, W3, W2, W1s, W3s, W2s)
    res = run_bass_kernel_spmd(nc, in_maps, list(range(N_CORES)), trace=_trace)
    _cached["last_res"] = res

    out = np.zeros((T, D), np.float32)
    for c in range(N_CORES):
        out += res.results[c]["partial"]
    for c in range(N_CORES):
        out[c * NSH:(c + 1) * NSH] += res.results[c]["y_shared"]
    return out.reshape(4, 2048, D).astype(np.float32)
